# revision 16
# baseline (speedup 1.0000x reference)
"""Trainium2 Bass kernel for nn_ArtNetwork (tiny MLP over 4M pixels).

Network: [N,2] -> Linear(2,16)+bias -> tanh -> 8x (Linear(16,16) -> tanh)
         -> Linear(16,3) -> sigmoid.

Strategy (pure data parallel over 8 NeuronCores):
  - Each core gets N/8 = 524288 points; weights replicated (packed host-side).
  - On-chip layout: 128 SBUF partitions = 8 point-groups x 16 hidden dims.
    Each hidden layer is ONE block-diagonal [128,128] bf16 matmul (8 copies
    of the 16x16 weight on the diagonal) per 512-column slice.
  - Input layer: x is DMA'd densely as interleaved (u,v) pairs, one group
    per partition; two K-accumulating matmuls with stride-2 rhs APs do the
    deinterleave for free.
  - Output layer: three channel matmuls (M=8, col tile_position 32q) write
    stride-3 PSUM columns so each partition ends up holding the final
    row-major (point, rgb) stream; one sigmoid pass; dense DMAs out.
  - tanh/sigmoid run on ScalarE directly PSUM->SBUF, 128 lanes, FD=2048.
    ScalarE is the roofline (~144 activation evals/point at 1/lane/cycle);
    batches are processed two-at-a-time interleaved so PE matmuls of one
    batch hide under ACT of the other.
"""

import numpy as np

N_POINTS = 4_194_304
N_CORES = 8
N_LOCAL = N_POINTS // N_CORES  # 524288
HIDDEN = 16
N_HID = 8
G = 8  # point-groups stacked on the 128 partitions
FD = 2048  # free-dim (points per group) per batch tile
MM_N = 512  # columns per matmul (one PSUM bank of fp32)
BATCH_PTS = G * FD  # 16384 points per batch tile

# dtype mode for the input-layer matmul: "f32r" (full-rate fp32) with
# fallback "f32" (4 cyc/row) or "bf16" (needs a DVE convert).
INPUT_MM = "f32r"


def _pack_weights(W_in, b_in, W_hidden, W_out):
    """Host-side packing into the lhsT forms the kernel needs.

    Returns dict of numpy arrays keyed by dram parameter name.
    lhsT convention: out = lhsT.T @ rhs.
    """
    import ml_dtypes

    W_in = np.asarray(W_in, np.float32)
    W_hidden = np.asarray(W_hidden, np.float32)
    W_out = np.asarray(W_out, np.float32)

    # input layer: rhs partition g holds group g's interleaved (u,v) stream.
    # mm_u uses even columns (u), mm_v odd columns (v), K=8, accumulating.
    w0 = np.zeros((8, 256), np.float32)
    for g in range(G):
        s = 16 * g
        w0[g, s : s + 16] = W_in[:, 0]
        w0[g, 128 + s : 128 + s + 16] = W_in[:, 1]

    # hidden layers: block-diagonal [128,128] per layer
    wh = np.zeros((128, 128 * N_HID), np.float32)
    for l in range(N_HID):
        for g in range(G):
            s = 16 * g
            wh[s : s + 16, 128 * l + s : 128 * l + s + 16] = W_hidden[l].T

    # output layer: channel c matmul maps group g -> out partition row g
    # (shifted by tile_position), i.e. lhsT_oc[16g+i, g] = W_out[c, i].
    # M=32 (columns g>=8 are zero) so the full 32-partition block is
    # written and the sigmoid never reads stale PSUM.
    wo = np.zeros((128, 96), np.float32)
    for c in range(3):
        for g in range(G):
            wo[16 * g : 16 * g + 16, 32 * c + g] = W_out[c]

    bias = np.tile(np.asarray(b_in, np.float32), G).reshape(128, 1)

    return {
        "w0": w0,
        "wh": np.asarray(wh, dtype=ml_dtypes.bfloat16),
        "wo": np.asarray(wo, dtype=ml_dtypes.bfloat16),
        "b": bias,
    }


def build_nc(n_local=N_LOCAL, fd=FD):
    """Build the Bass graph for one core's shard of n_local points."""
    from contextlib import ExitStack

    import concourse.bass as bass  # noqa: F401
    import concourse.tile as tile
    from concourse import bacc, mybir

    assert n_local % (G * fd) == 0
    n_batches = n_local // (G * fd)
    assert fd % MM_N == 0
    nq = fd // MM_N

    F32 = mybir.dt.float32
    F32R = mybir.dt.float32r
    BF16 = mybir.dt.bfloat16
    XDT = {"f32r": F32R, "f32": F32, "bf16": BF16}[INPUT_MM]
    X_IN_DT = F32R if INPUT_MM == "f32r" else F32
    TANH = mybir.ActivationFunctionType.Tanh
    SIGM = mybir.ActivationFunctionType.Sigmoid

    nc = bacc.Bacc(
        "TRN2",
        target_bir_lowering=False,
        debug=False,
        enable_asserts=False,
        num_devices=N_CORES,
    )

    x_d = nc.dram_tensor("x", [n_local, 2], X_IN_DT, kind="ExternalInput").ap()
    w0_d = nc.dram_tensor("w0", [8, 256], X_IN_DT, kind="ExternalInput").ap()
    wh_d = nc.dram_tensor("wh", [128, 128 * N_HID], BF16, kind="ExternalInput").ap()
    wo_d = nc.dram_tensor("wo", [128, 96], BF16, kind="ExternalInput").ap()
    b_d = nc.dram_tensor("b", [128, 1], F32, kind="ExternalInput").ap()
    y_d = nc.dram_tensor("out", [n_local, 3], F32, kind="ExternalOutput").ap()

    with tile.TileContext(nc) as tc, ExitStack() as ctx:
        wpool = ctx.enter_context(tc.tile_pool(name="wpool", bufs=1))
        xpool = ctx.enter_context(tc.tile_pool(name="xpool", bufs=3))
        hpool = ctx.enter_context(tc.tile_pool(name="hpool", bufs=6))
        opool = ctx.enter_context(tc.tile_pool(name="opool", bufs=3))
        pspool = ctx.enter_context(tc.tile_pool(name="pspool", bufs=2, space="PSUM"))

        W0 = wpool.tile([8, 256], X_IN_DT)
        nc.sync.dma_start(out=W0[:], in_=w0_d[:, :])
        WH = wpool.tile([128, 128 * N_HID], BF16)
        nc.sync.dma_start(out=WH[:], in_=wh_d[:, :])
        WO = wpool.tile([128, 96], BF16)
        nc.sync.dma_start(out=WO[:], in_=wo_d[:, :])
        B = wpool.tile([128, 1], F32)
        nc.sync.dma_start(out=B[:], in_=b_d[:, :])

        assert n_batches % 2 == 0
        for pair in range(n_batches // 2):
            bb = (2 * pair, 2 * pair + 1)
            xb = {}
            for b in bb:
                # group g's (u,v) interleaved stream, densely loaded
                xuv = xpool.tile([8, 2 * fd], X_IN_DT)
                src = x_d[b * G * fd : (b + 1) * G * fd, :].rearrange(
                    "(g f) c -> g (f c)", g=G
                )
                nc.sync.dma_start(out=xuv[:], in_=src)
                xb[b] = xuv

            cur = {}
            # input layer: two accumulating matmuls (u from even cols,
            # v from odd cols), K=8
            for b in bb:
                pst = pspool.tile([128, max(fd, 2048)], F32, tag="ps")
                ps = pst[:, 0:fd]
                for q in range(nq):
                    c0 = 2 * q * MM_N
                    nc.tensor.matmul(
                        out=ps[:, q * MM_N : (q + 1) * MM_N],
                        lhsT=W0[:, 0:128],
                        rhs=xb[b][:, c0 : c0 + 2 * MM_N : 2],
                        start=True,
                        stop=False,
                    )
                    nc.tensor.matmul(
                        out=ps[:, q * MM_N : (q + 1) * MM_N],
                        lhsT=W0[:, 128:256],
                        rhs=xb[b][:, c0 + 1 : c0 + 2 * MM_N : 2],
                        start=False,
                        stop=True,
                    )
                h = hpool.tile([128, fd], BF16)
                nc.scalar.activation(h[:], ps[:], TANH, bias=B[:, 0:1])
                cur[b] = h

            # hidden layers
            for l in range(N_HID):
                for b in bb:
                    ps = pspool.tile([128, fd], F32)
                    for q in range(nq):
                        nc.tensor.matmul(
                            out=ps[:, q * MM_N : (q + 1) * MM_N],
                            lhsT=WH[:, 128 * l : 128 * (l + 1)],
                            rhs=cur[b][:, q * MM_N : (q + 1) * MM_N],
                            start=True,
                            stop=True,
                        )
                    h = hpool.tile([128, fd], BF16)
                    nc.scalar.activation(h[:], ps[:], TANH)
                    cur[b] = h

            # output layer: channel matmuls write stride-3 psum columns so
            # partition 32q+g ends up holding the row-major rgb stream of
            # q-slice q of group g. A matmul output may not cross a PSUM
            # bank (512 f32), so each 512-point q-slice is split into 4
            # sub-blocks of SUB=128 points (384 f32 + 128 pad per bank).
            SUB = MM_N // 4
            for b in bb:
                pst = pspool.tile([128, max(fd, 2048)], F32, tag="ps")
                po = pst[:, 0:2048]
                for q in range(nq):
                    for s in range(4):
                        for ch in range(3):
                            nc.tensor.matmul(
                                out=po[
                                    32 * q : 32 * q + 32,
                                    512 * s + ch : 512 * s + 3 * SUB : 3,
                                ],
                                lhsT=WO[:, 32 * ch : 32 * ch + 32],
                                rhs=cur[b][
                                    :, q * MM_N + s * SUB : q * MM_N + (s + 1) * SUB
                                ],
                                start=(ch == 0),
                                stop=(ch == 2),
                                tile_position=(0, 32 * q),
                            )
                o = opool.tile([128, 3 * MM_N], F32)
                np_used = 32 * nq
                po_v = po.rearrange("p (s j) -> p s j", s=4)[
                    0:np_used, :, 0 : 3 * SUB
                ]
                o_v = o.rearrange("p (s j) -> p s j", s=4)[0:np_used]
                nc.scalar.activation(o_v, po_v, SIGM)
                dst_all = y_d[b * G * fd : (b + 1) * G * fd, :].rearrange(
                    "(g f) c -> g (f c)", g=G
                )
                for q in range(nq):
                    nc.sync.dma_start(
                        out=dst_all[:, 3 * q * MM_N : 3 * (q + 1) * MM_N],
                        in_=o[32 * q : 32 * q + 8, :],
                    )

    nc.compile()
    return nc


_cache = {}


def _get_nc(n_local=N_LOCAL, fd=FD):
    key = (n_local, fd)
    if key not in _cache:
        _cache[key] = build_nc(n_local, fd)
    return _cache[key]


def _in_maps(x, W_in, b_in, W_hidden, W_out, n_local=N_LOCAL):
    packed = _pack_weights(W_in, b_in, W_hidden, W_out)
    x = np.asarray(x, np.float32)
    maps = []
    for c in range(N_CORES):
        m = dict(packed)
        m["x"] = np.ascontiguousarray(x[c * n_local : (c + 1) * n_local])
        maps.append(m)
    return maps


def kernel(x, W_in, b_in, W_hidden, W_out):
    from concourse.bass_utils import run_bass_kernel_spmd

    nc = _get_nc()
    maps = _in_maps(x, W_in, b_in, W_hidden, W_out)
    res = run_bass_kernel_spmd(nc, maps, core_ids=list(range(N_CORES)))
    return np.concatenate([res.results[c]["out"] for c in range(N_CORES)], axis=0)


def make_runner(nc, in_maps):
    """Build a reusable sharded executor with device-resident inputs.

    Returns (run, n_params) where run() executes the NEFF once on all 8
    cores and returns the jax output arrays (call jax.block_until_ready).
    Mirrors bass2jax.run_bass_via_pjrt but keeps inputs on device and does
    NOT donate outputs, so repeated calls are cheap.
    """
    import jax
    from jax.sharding import Mesh, PartitionSpec
    from jax.experimental.shard_map import shard_map

    from concourse import bass2jax, mybir

    bass2jax.install_neuronx_cc_hook()
    n_cores = len(in_maps)
    partition_name = nc.partition_id_tensor.name if nc.partition_id_tensor else None

    in_names, out_names, out_avals, zero_outs = [], [], [], []
    for alloc in nc.m.functions[0].allocations:
        if not isinstance(alloc, mybir.MemoryLocationSet):
            continue
        name = alloc.memorylocations[0].name
        if alloc.kind == "ExternalInput":
            if name != partition_name:
                in_names.append(name)
        elif alloc.kind == "ExternalOutput":
            shape = tuple(alloc.tensor_shape)
            dtype = mybir.dt.np(alloc.dtype)
            out_names.append(name)
            out_avals.append(jax.core.ShapedArray(shape, dtype))
            zero_outs.append(np.zeros(shape, dtype))
    n_params = len(in_names)
    all_names = in_names + out_names
    if partition_name is not None:
        all_names = all_names + [partition_name]

    def _body(*args):
        operands = list(args)
        if partition_name is not None:
            operands.append(bass2jax.partition_id_tensor())
        outs = bass2jax._bass_exec_p.bind(
            *operands,
            out_avals=tuple(out_avals),
            in_names=tuple(all_names),
            out_names=tuple(out_names),
            lowering_input_output_aliases=(),
            sim_require_finite=True,
            sim_require_nnan=True,
            nc=nc,
        )
        return tuple(outs)

    devices = jax.devices()[:n_cores]
    mesh = Mesh(np.asarray(devices), ("core",))
    n_out = len(out_names)
    sharded = jax.jit(
        shard_map(
            _body,
            mesh=mesh,
            in_specs=(PartitionSpec("core"),) * (n_params + n_out),
            out_specs=(PartitionSpec("core"),) * n_out,
            check_rep=False,
        ),
        keep_unused=True,
    )
    sh = jax.sharding.NamedSharding(mesh, PartitionSpec("core"))
    concat_in = [
        np.concatenate([np.asarray(in_maps[c][k]) for c in range(n_cores)], axis=0)
        for k in in_names
    ]
    concat_zero = [
        np.zeros((n_cores * z.shape[0], *z.shape[1:]), z.dtype) for z in zero_outs
    ]
    dev_args = [jax.device_put(a, sh) for a in concat_in + concat_zero]

    def run():
        return sharded(*dev_args)

    return run, out_names


def run_timed(x, W_in, b_in, W_hidden, W_out, iters=12):
    """Returns (output, wall_times_s list, out_names)."""
    import time

    import jax

    nc = _get_nc()
    maps = _in_maps(x, W_in, b_in, W_hidden, W_out)
    run, out_names = make_runner(nc, maps)
    outs = run()
    jax.block_until_ready(outs)
    times = []
    for _ in range(iters):
        t0 = time.perf_counter()
        o = run()
        jax.block_until_ready(o)
        times.append(time.perf_counter() - t0)
    full = np.asarray(outs[out_names.index("out")])
    out = full.reshape(N_CORES, -1, 3).reshape(-1, 3)
    return out, times, out_names
